# revision 105
# baseline (speedup 1.0000x reference)
"""AdSBHNet integral kernel for 8 TRN2 NeuronCores — 2-up packed layout.

Math (all-real reformulation of the complex reference):
  poly(c,z) = sum_{i=1..5} c_i z^i ;  f = (1-z^4) e^{poly(a,z)} ; g = e^{poly(b,z)}/(1-z^4)
  z = zs*u.
  L: w = A/(D+eps(1+i)) - 1 + eps(1+i),  A = zs^4 f(z), Dre = z^4 f(zs)+eps
     integrand = sqrt(g)/sqrt(w);  L = (2/pi) * zs * sum_j(wt_j * integrand_j)
  V: inner = 1 - Y/(X+eps(1+i)) + eps(1+i), Y = z^4 f(zs), X = zs^4 f(z)
     term = sqrt(f g)/sqrt(inner) - 1; integrand = term/(z^2+eps(1+i))
     V = 2pi*zs*sum_j(wt_j integrand_j) - 2pi/zs
  Complex sqrt of w=re+i*im with r=|w|: sqrt(g)/sqrt(w) has real part
  sqrt(G*(r+re)) and imag magnitude sqrt(G*(r-re)), G = g/(2 r^2); computed in
  log domain with rlarge = r+|re| and the small branch via
  sqrt(G*rsmall) = sqrt(G*rlarge)*|im|/rlarge, routed by sign(re) via max/min.

Quadrature: 64-node mixed rules replicating the reference's 2000/1500-pt
trapezoid sums (L: 32 GL + last 32 trapezoid pts; V: first 24 pts + 24 GL +
last 16 pts), validated ~5e-4 relative against the reference in float64.

Layout (2-up packing): 64 u-nodes x 2 batch-halves fill the 128 partitions;
partition p = node (p%64) for batch half (p//64); the free dim holds 512 of
the core's 1024 rows. Each integral's pass is split into two independent
256-column chunks, and the four (L/V x chunk) chains are emitted as
interleaved generators (chunk 0 staggered a few ops ahead) so the in-order
engine queues always hold ready work from another chain. Per-batch rows live
in [2,512] "half" layout through setup (itself column-chunked); broadcasts
to [128,*] use K=2 matmuls; poly(z)+k*ln(zs) come from K=14 matmuls whose
two extra contraction rows carry ln(zs) per half (filled by sbuf-to-sbuf
DMA). The weighted node-sum is a K=128 bf16 matmul with [128,2] half-masked
weights; -2pi/zs accumulates into the V.re PSUM via a tiny K=2 matmul
paired adjacently with the main reduction (PSUM accumulation groups on one
bank must not interleave). A short chain of tiny PE warmup matmuls at t=0
ramps the tensor-engine p-state before the setup matmuls. Engine placement
balances DVE (customs/recips/routing), Activation (exp/ln/square/copies),
and Pool (plain mult/add tensor-tensor) per the TimelineSim cost model.

Sharding: pure data parallel, zs batch split 8 ways; a/b replicated.
"""

import math
import sys

import numpy as np

sys.path.insert(0, "/opt/trn_rl_repo")

import concourse.bass as bass
import concourse.bacc as bacc
import concourse.mybir as mybir
from concourse import bass_utils
from concourse.tile import TileContext

F32 = mybir.dt.float32
I32 = mybir.dt.int32
BF16 = mybir.dt.bfloat16
U16 = mybir.dt.uint16
OP = mybir.AluOpType
AF = mybir.ActivationFunctionType

EPS = 1e-6
EPS2 = EPS * EPS
NU_L = 2000
NU_V = 1500
B = 8192
NCORES = 8
BLOC = B // NCORES       # 1024 rows per core
HALF = BLOC // 2         # 512 rows per half
H_L = (1.0 - 2 * EPS) / (NU_L - 1)
H_V = (1.0 - 2 * EPS) / (NU_V - 1)
LN2 = math.log(2.0)
NN = 64                  # u-nodes per integral (2-up packed into 128)


def _mixed_nodes(Nu, h, n_head, n_gl, n_tail):
    """Nodes/weights replicating the Nu-pt trapezoid sum h*(f0/2+...+fN/2)
    with Gauss-Legendre on the smooth middle (float64)."""
    u = EPS + h * np.arange(Nu)
    nodes, wts = [], []
    if n_head:
        nodes.append(u[: n_head + 1])
        w = np.full(n_head + 1, h)
        w[0] = w[-1] = h / 2
        wts.append(w)
    lo = u[n_head]
    hi = u[Nu - 1 - n_tail]
    x, w = np.polynomial.legendre.leggauss(n_gl)
    nodes.append(0.5 * (hi + lo) + 0.5 * (hi - lo) * x)
    wts.append(0.5 * (hi - lo) * w)
    nodes.append(u[Nu - 1 - n_tail:])
    wt = np.full(n_tail + 1, h)
    wt[0] = wt[-1] = h / 2
    wts.append(wt)
    return np.concatenate(nodes), np.concatenate(wts)


_UL, _WL = _mixed_nodes(NU_L, H_L, 0, 32, 31)      # 64 nodes
_UV, _WV = _mixed_nodes(NU_V, H_V, 23, 24, 15)     # 64 nodes
assert len(_UL) == NN and len(_UV) == NN

_K6 = np.arange(6.0)


def _upow14(u, lncoef):
    """[14,128] lhsT: rows 0-5 = u^k masked to half0, rows 6-11 = u^k masked
    to half1, rows 12/13 = lncoef masked per half (k*ln(zs) terms; the rhs
    rows 12/13 hold plain ln(zs) per half)."""
    out = np.zeros((14, 128), dtype=np.float64)
    up = u[None, :] ** _K6[:, None]          # [6,64]
    out[0:6, 0:64] = up
    out[6:12, 64:128] = up
    out[12, 0:64] = lncoef
    out[13, 64:128] = lncoef
    return out


def _halfmask(w):
    """[128,2]: col0 = w on half0 partitions, col1 = w on half1."""
    out = np.zeros((128, 2), dtype=np.float64)
    out[0:64, 0] = w
    out[64:128, 1] = w
    return out


# ---- packed constants ------------------------------------------------------
_UPOW_LA = _upow14(_UL, 4.0)                 # pa(z) + ln zs^4
_UPOW_LB = _upow14(_UL, 2.0)                 # pb(z) + 2 ln zs  (zs*sqrt(g))
_UPOW_VA = _upow14(_UV, 4.0)                 # pa(z) + ln zs^4
_UPOW_VAB = _upow14(_UV, 2.0)                # pa+pb + 2 ln zs  (zs*sqrt(fg))

_SEL2 = np.zeros((2, 128))                   # broadcast selector
_SEL2[0, 0:64] = 1.0
_SEL2[1, 64:128] = 1.0
_SELDUP = np.zeros((2, 12))                  # lnz2 -> 12 rows (6 per half)
_SELDUP[0, 0:6] = 1.0
_SELDUP[1, 6:12] = 1.0

_WRED_L = np.concatenate(                    # [128,4]: Lre cols 0:2, Lim 2:4
    [_halfmask((2.0 / math.pi) * _WL), _halfmask(-(2.0 / math.pi) * _WL)], axis=1)
_WRED_V = np.concatenate(
    [_halfmask(2.0 * math.pi * _WV), _halfmask(-2.0 * math.pi * _WV)], axis=1)

_NODEC = np.stack(                            # [128,5] per-partition consts
    [np.tile(_UL**4, 2), np.tile(_UV**4, 2), np.tile(_UV**2, 2),
     np.tile(-(_UL**4), 2), np.tile(-(_UV**4), 2)], axis=1)

_KCOL12 = np.tile(_K6, 2).reshape(12, 1)      # [12,1] exponents
_DIAG2 = -2.0 * math.pi * np.eye(2)           # [2,2] V_disc accumulate

# mega-const: lhsT consts packed side by side (see slices in build_nc)
_CONST = np.zeros((128, 800), dtype=np.float64)
_CONST[0:14, 0:128] = _UPOW_LA
_CONST[0:14, 128:256] = _UPOW_LB
_CONST[0:14, 256:384] = _UPOW_VA
_CONST[0:14, 384:512] = _UPOW_VAB
_CONST[0:2, 512:640] = _SEL2
_CONST[0:2, 640:652] = _SELDUP
_CONST[0:12, 652:653] = _KCOL12
_CONST[0:2, 653:655] = _DIAG2
_CONST[:, 655:660] = _NODEC
_CONST[:, 660:664] = _WRED_L
_CONST[:, 664:668] = _WRED_V
_CONST[64, 668:732] = 1.0                     # ones row at partition 64
_CONST_F32 = _CONST.astype(np.float32)

_CSM = np.zeros((12, 397), dtype=np.float64)   # early setup consts
_CSM[0:2, 0:12] = _SELDUP
_CSM[0:2, 12:140] = _SEL2
_CSM[0:12, 140:141] = _KCOL12
_CSM[0:2, 141:269] = 4.0 * _SEL2               # + ln zs^4 accum
_CSM[0:2, 269:397] = 2.0 * _SEL2               # + 2 ln zs accum
_CSM_F32 = _CSM.astype(np.float32)

# ---- custom DVE ops (registered into concourse.dve_ops at import) ---------
import concourse.dve_ops as _dops
from concourse.dve_spec import C0 as _C0
from concourse.dve_spec import C1 as _C1
from concourse.dve_spec import C2 as _C2
from concourse.dve_spec import One as _One
from concourse.dve_spec import Spec as _Spec
from concourse.dve_spec import Src0 as _Src0
from concourse.dve_spec import Src1 as _Src1
from concourse.dve_spec import Zero as _Zero
from concourse.dve_spec import _has_src1 as _hs1
from concourse.dve_spec import lower as _dve_lower
from concourse.dve_spec import maxx as _maxx
from concourse.dve_spec import select as _select
from concourse.dve_spec import sq as _sq
from concourse.dve_uop import DveOpSpec as _DveOpSpec


def _register_dve(name, spec):
    for op in _dops.OPS:
        if op.name == name:
            return op
    row = _dops._CUSTOM_DVE_ROW_BASE + len(_dops.OPS)
    assert row < 0x20
    _dops._SUB_OPCODE_FOR_NAME[name] = row
    shas = {}
    for ver in ("v3", "v4"):
        tmp = _DveOpSpec(name=name, opcode=row, uops=_dve_lower(spec, ver=ver),
                         rd1_en=_hs1(spec))
        shas[ver] = tmp.sha(ver)
    op = _dops.DveOp(name, spec, subdim=False, uops_sha=shas)
    _dops.OPS.append(op)
    return op


# out = (in0*s0 + s1)^2 + imm2   (n2 = (c1*u4+eps)^2+eps^2 etc.)
_AFFSQ = _register_dve("ANT_AFFSQ", _Spec(
    body=_sq(_Src0 * _C0 + _C1) + _C2,
    reference=lambda in0, in1, s0, s1, imm2: (in0 * s0 + s1) ** 2 + imm2,
))
# out = in0*(in1*s0 + s1)       (tDp = t*(c1*u4+eps); t2 = rn2v*(c1*u4))
_MULAFF = _register_dve("ANT_MULAFF", _Spec(
    body=_Src0 * (_Src1 * _C0 + _C1),
    reference=lambda in0, in1, s0, s1, imm2: in0 * (in1 * s0 + s1),
))
# out = (in0-s0)^2 + imm2*(1-in1)^2   (L: r2 = re^2 + im^2 from tDp,t)
_R2FULL = _register_dve("ANT_R2FULL", _Spec(
    body=_sq(_Src0 - _C0) + _sq(_One - _Src1) * _C2,
    reference=lambda in0, in1, s0, s1, imm2: (in0 - s0) ** 2 + imm2 * (1.0 - in1) ** 2,
))
# out = (in0-s0)^2 + imm2*(1+in1)^2   (V variant: im2 = eps^2 (1+t2)^2)
_R2FULLP = _register_dve("ANT_R2FULLP", _Spec(
    body=_sq(_Src0 - _C0) + _sq(_One + _Src1) * _C2,
    reference=lambda in0, in1, s0, s1, imm2: (in0 - s0) ** 2 + imm2 * (1.0 + in1) ** 2,
))
# out = |in0-s0| + in1          (rlg = |re| + r)
_ABSD_ADD = _register_dve("ANT_ABSDADD", _Spec(
    body=_maxx(_Src0 - _C0, _C0 - _Src0) + _Src1,
    reference=lambda in0, in1, s0, s1, imm2: np.abs(in0 - s0) + in1,
))
# out = +in0 where in1 <= s0 else -in0
_SGN_LE = _register_dve("ANT_SGNLE", _Spec(
    body=_select(_Src1 <= _C0, _Src0, _Zero - _Src0),
    reference=lambda in0, in1, s0, s1, imm2: np.where(in1 <= s0, in0, -in0),
))
# out = +in0 where in1 >= s0 else -in0
_SGN_GE = _register_dve("ANT_SGNGE", _Spec(
    body=_select(_Src1 >= _C0, _Src0, _Zero - _Src0),
    reference=lambda in0, in1, s0, s1, imm2: np.where(in1 >= s0, in0, -in0),
))
# out = |in0| * sign-ish(s0 - in1): +|in0| where in1 <= s0 else -|in0|
_ABS_SGNLE = _register_dve("ANT_ABSSGNLE", _Spec(
    body=_select(_Src1 <= _C0, _maxx(_Src0, _Zero - _Src0),
                 _Zero - _maxx(_Src0, _Zero - _Src0)),
    reference=lambda in0, in1, s0, s1, imm2: np.where(in1 <= s0, np.abs(in0),
                                                      -np.abs(in0)),
))
# out = in1*(1-in1)*in0          (c1 = zs4*(1-zs4)*e^pa(zs))
_C1ROW = _register_dve("ANT_C1ROW", _Spec(
    body=_Src1 * (_One - _Src1) * _Src0,
    reference=lambda in0, in1, s0, s1, imm2: in1 * (1.0 - in1) * in0,
))
# out = |1-in1| * in0 * s0       (L small-branch: h1 = eps|1-t|/rlarge)
_ABS1M_MUL = _register_dve("ANT_ABS1MMUL", _Spec(
    body=_maxx(_One - _Src1, _Src1 - _One) * _Src0 * _C0,
    reference=lambda in0, in1, s0, s1, imm2: np.abs(1.0 - in1) * in0 * s0,
))
# out = (1+in1) * in0 * s0       (V small-branch: h1v = eps(1+t2)/rlarge)
_ONEP_MUL = _register_dve("ANT_ONEPMUL", _Spec(
    body=(_One + _Src1) * _Src0 * _C0,
    reference=lambda in0, in1, s0, s1, imm2: (1.0 + in1) * in0 * s0,
))


def build_nc(reps=1):
    nc = bacc.Bacc("TRN2", target_bir_lowering=False, debug=False, num_devices=NCORES)
    a_d = nc.declare_dram_parameter("a", [5], F32, isOutput=False)
    b_d = nc.declare_dram_parameter("b", [5], F32, isOutput=False)
    zs_d = nc.declare_dram_parameter("zs", [BLOC], F32, isOutput=False)
    out_d = nc.declare_dram_parameter("out", [4, BLOC], F32, isOutput=True)

    const_d = nc.inline_tensor(_CONST_F32, name="cpack")
    csm_d = nc.inline_tensor(_CSM_F32, name="csmall")

    with TileContext(nc) as tc:
        with (
            tc.tile_pool(name="cst", bufs=1) as cst,
            tc.tile_pool(name="wk", bufs=1) as wk,
            tc.tile_pool(name="ps", bufs=1, space="PSUM") as pspool,
        ):
            v = nc.vector
            sc = nc.scalar
            gp = nc.gpsimd
            mm = nc.tensor.matmul

            def W(tag, dt=F32, shape=(128, HALF)):
                return wk.tile(list(shape), dt, tag=tag, name=f"t{tag}")

            # ---------------- constants ----------------
            # zs gates the whole setup chain: give it its own (DVE) DMA queue
            # so it doesn't serialize behind the big const transfer on SP
            zrow2 = cst.tile([2, HALF], F32)
            nc.gpsimd.dma_start(out=zrow2[:], in_=zs_d[:].rearrange("(o n) -> o n", o=2))
            csm = cst.tile([12, 397], F32)
            nc.sync.dma_start(out=csm[:], in_=csm_d[:, :])
            cpk = cst.tile([128, 800], F32)
            nc.sync.dma_start(out=cpk[:], in_=const_d[:, :])

            mmr = mm

            lhs_LA = cpk[0:14, 0:128]
            lhs_LB = cpk[0:14, 128:256]
            lhs_VA = cpk[0:14, 256:384]
            lhs_VAB = cpk[0:14, 384:512]
            sel2 = csm[0:2, 12:140]
            seldup = csm[0:2, 0:12]
            kcol12 = csm[0:12, 140:141]
            sel2x4 = csm[0:2, 141:269]
            sel2x2 = csm[0:2, 269:397]

            # PE pstate warmup: keep the tensor engine continuously busy
            # through the DMA/ln setup so the critical matmuls run at full
            # clock (cost model: >3us continuous busy -> max speed). Operand
            # is a memset tile so warmups start before any DMA lands.
            wm_in = cst.tile([1, 64], F32)
            v.memset(wm_in[:], 1.0)
            wps = pspool.tile([128, HALF], F32, tag="paL", name="warm")
            for _ in range(12):
                mm(wps[0:1, 0:64], wm_in[0:1, 0:1], wm_in[:], start=True,
                   stop=True)
            diag2 = cpk[0:2, 653:655]
            u4L_c = cpk[:, 655:656]
            u4V_c = cpk[:, 656:657]
            u2V_c = cpk[:, 657:658]
            nu4L_c = cpk[:, 658:659]
            nu4V_c = cpk[:, 659:660]
            wred = cst.tile([128, 8], BF16)
            v.tensor_copy(wred[:], cpk[:, 660:668])
            wLre_c = wred[:, 0:2]
            wLim_c = wred[:, 2:4]
            wVre_c = wred[:, 4:6]
            wVim_c = wred[:, 6:8]

            nhln2 = cst.tile([128, 1], F32)
            v.memset(nhln2[:], -0.5 * LN2)
            c_eps = cst.tile([128, 1], F32)
            v.memset(c_eps[:], EPS)

            # a/b coefficient columns [12,1], duplicated per batch half
            aext = cst.tile([12, 1], F32)
            bext = cst.tile([12, 1], F32)
            v.memset(aext[:], 0.0)
            v.memset(bext[:], 0.0)
            nc.sync.dma_start(out=aext[1:6, 0:1], in_=a_d[:])
            nc.sync.dma_start(out=aext[7:12, 0:1], in_=a_d[:])
            nc.sync.dma_start(out=bext[1:6, 0:1], in_=b_d[:])
            nc.sync.dma_start(out=bext[7:12, 0:1], in_=b_d[:])
            abext = cst.tile([12, 1], F32)
            v.tensor_tensor(abext[:], aext[:], bext[:], OP.add)
            # caext [12,2]: col h = a-coeffs on rows 6h+1..6h+5 (pa(zs) lhsT)
            caext = cst.tile([12, 2], F32)
            v.memset(caext[:], 0.0)
            nc.sync.dma_start(out=caext[1:6, 0:1], in_=a_d[:])
            nc.sync.dma_start(out=caext[7:12, 1:2], in_=a_d[:])

            # ---------------- setup (rows in [2,512] half layout) ----------
            # All row/broadcast work is column-chunked (2 x 256) so the first
            # chunk's pass work starts while chunk 1's setup is in flight.
            CH = HALF // 2
            CHUNKS = [slice(0, CH), slice(CH, 2 * CH)]

            lnz2 = cst.tile([2, HALF], F32)
            zpow12 = cst.tile([12, HALF], F32)
            zs2_2 = cst.tile([2, HALF], F32)
            zs4_2 = cst.tile([2, HALF], F32)
            e_pazs = cst.tile([2, HALF], F32)
            c1_2 = cst.tile([2, HALF], F32)
            rhs_pa = cst.tile([14, HALF], F32)
            rhs_pb = cst.tile([14, HALF], F32)
            rhs_pab = cst.tile([14, HALF], F32)
            invz2 = cst.tile([2, HALF], F32)  # filled late, in v_pass
            zs4b = cst.tile([128, HALF], F32)
            ps12 = pspool.tile([12, HALF], F32, tag="ps12", name="ps12")
            c1b = pspool.tile([128, HALF], F32, tag="c1b", name="c1b")
            zs2b = pspool.tile([128, HALF], F32, tag="zs2b", name="zs2b")
            zsb = pspool.tile([128, HALF], F32, tag="zsb", name="zsb")

            def setup_chunk(s):
                sc.activation(lnz2[:, s], zrow2[:, s], AF.Ln)
                mm(ps12[0:12, s], seldup, lnz2[:, s], start=True, stop=True)
                pe_fill(2)
                sc.activation(zpow12[:, s], ps12[0:12, s], AF.Exp,
                              scale=kcol12)
                gp.tensor_tensor(zs2_2[:, s], zrow2[:, s], zrow2[:, s],
                                 OP.mult)
                gp.tensor_tensor(zs4_2[:, s], zs2_2[:, s], zs2_2[:, s],
                                 OP.mult)
                nc.sync.dma_start(out=rhs_pa[12:14, s], in_=lnz2[:, s])
                nc.sync.dma_start(out=rhs_pb[12:14, s], in_=lnz2[:, s])
                nc.sync.dma_start(out=rhs_pab[12:14, s], in_=lnz2[:, s])
                v.tensor_scalar(rhs_pa[0:12, s], zpow12[:, s], aext[:, 0:1],
                                None, OP.mult)
                v.tensor_scalar(rhs_pb[0:12, s], zpow12[:, s], bext[:, 0:1],
                                None, OP.mult)
                v.tensor_scalar(rhs_pab[0:12, s], zpow12[:, s], abext[:, 0:1],
                                None, OP.mult)
                mm(ps12[0:2, s], caext[:], zpow12[:, s], start=True,
                   stop=True)
                pe_fill(2)
                sc.activation(e_pazs[:, s], ps12[0:2, s], AF.Exp)
                v._custom_dve(_C1ROW, out=c1_2[:, s], in0=e_pazs[:, s],
                              in1=zs4_2[:, s])
                mm(c1b[:, s], sel2, c1_2[:, s], start=True, stop=True)
                mm(zs2b[:, s], sel2, zs2_2[:, s], start=True, stop=True)
                mm(zsb[:, s], sel2, zrow2[:, s], start=True, stop=True)
                pe_fill(2)
                sc.activation(zs4b[:, s], zs2b[:, s], AF.Square)

            setup_chunk(CHUNKS[0])
            setup_chunk(CHUNKS[1])

            # output staging sbuf [2, 2048]: row = batch half, free = 4x512
            # (Lre | Lim | Vre | Vim); DMA rearranges to out[4,1024]
            osb = cst.tile([2, 4 * HALF], F32)

            # ---------------- L / V passes (4 interleaved chunk chains) -----
            # Generators yield after each emitted instruction; round-robin
            # emission keeps every engine queue fed with independent work
            # from the other chains while one chain waits on a cross-engine
            # dep. Shared psum tiles are accessed by column slice.
            shared = {}

            def P(name, tag=None):
                if name not in shared:
                    shared[name] = pspool.tile([128, HALF], F32,
                                               tag=tag or name, name=name)
                return shared[name]

            def l_pass(i, s):
                def W(tag, dt=F32):
                    return wk.tile([128, CH], dt, tag=f"{tag}c{i}",
                                   name=f"t{tag}c{i}")

                pa_L = P("pa_L", "paL")
                mm(pa_L[:, s], lhs_LA, rhs_pa[:, s], start=True, stop=True)
                pb_L = P("pb_L", "pbL")
                mm(pb_L[:, s], lhs_LB, rhs_pb[:, s], start=True, stop=True)
                yield
                e_a2 = W("u01")
                sc.activation(e_a2[:], pa_L[:, s], AF.Exp)   # zs^4 e^{pa(z)}
                yield
                omz4 = W("u02")
                sc.activation(omz4[:], zs4b[:, s], AF.Copy, bias=1.0,
                              scale=nu4L_c)
                yield
                X = W("u03")
                gp.tensor_tensor(X[:], omz4[:], e_a2[:], OP.mult)

                yield
                n2 = W("u04")
                v._custom_dve(_AFFSQ, out=n2[:], in0=c1b[:, s], s0=u4L_c,
                              s1=EPS, imm2=EPS2)             # |D|^2
                yield
                rn2 = W("u05")
                v.reciprocal_approx_fast(rn2[:], n2[:])
                yield
                t_ = W("u06")
                v.tensor_tensor(t_[:], X[:], rn2[:], OP.mult)
                yield
                tDp = W("u07")
                v._custom_dve(_MULAFF, out=tDp[:], in0=t_[:], in1=c1b[:, s],
                              s0=u4L_c, s1=EPS)
                yield
                r2s = W("u08")
                v._custom_dve(_R2FULL, out=r2s[:], in0=tDp[:], in1=t_[:],
                              s0=1.0 - EPS, imm2=EPS2)
                yield
                lnom = W("u09")
                sc.activation(lnom[:], omz4[:], AF.Ln)
                yield
                lnr2s = W("u10")
                sc.activation(lnr2s[:], r2s[:], AF.Ln)
                yield
                r_ = W("u11", dt=BF16)
                sc.activation(r_[:], lnr2s[:], AF.Exp, scale=0.5)
                yield
                rlg = W("u12")
                v._custom_dve(_ABSD_ADD, out=rlg[:], in0=tDp[:], in1=r_[:],
                              s0=1.0 - EPS)
                yield
                lnrlg = W("u13")
                sc.activation(lnrlg[:], rlg[:], AF.Ln)
                yield
                base = W("u14")
                v.tensor_tensor(base[:], pb_L[:, s], lnom[:], OP.subtract)
                yield
                base2 = W("u15")
                gp.tensor_tensor(base2[:], base[:], lnr2s[:], OP.subtract)
                yield
                lnglg = W("u16")
                gp.tensor_tensor(lnglg[:], base2[:], lnrlg[:], OP.add)
                yield
                SS = W("u17", dt=BF16)
                sc.activation(SS[:], lnglg[:], AF.Exp, bias=nhln2[:, 0:1],
                              scale=0.5)
                yield
                rcpl = W("u18")
                v.reciprocal_approx_fast(rcpl[:], rlg[:])
                yield
                h1 = W("u19", dt=BF16)
                v._custom_dve(_ABS1M_MUL, out=h1[:], in0=rcpl[:], in1=t_[:],
                              s0=EPS)
                yield
                TTs = W("u20", dt=BF16)
                v.tensor_tensor(TTs[:], SS[:], h1[:], OP.mult)
                yield
                SSs = W("u21", dt=BF16)
                v._custom_dve(_SGN_GE, out=SSs[:], in0=SS[:], in1=tDp[:],
                              s0=1.0 - EPS)
                yield
                igre = W("u22", dt=BF16)
                v.tensor_tensor(igre[:], SSs[:], TTs[:], OP.max)
                yield
                mn = W("u23", dt=BF16)
                v.tensor_tensor(mn[:], SSs[:], TTs[:], OP.min)
                yield
                igqs = W("u24", dt=BF16)
                v._custom_dve(_ABS_SGNLE, out=igqs[:], in0=mn[:], in1=t_[:],
                              s0=1.0)
                yield
                redLre = P("redLre", "c1b")
                mm(redLre[0:2, s], wLre_c, igre[:], start=True, stop=True)
                redLim = P("redLim", "zs2b")
                mm(redLim[0:2, s], wLim_c, igqs[:], start=True, stop=True)
                yield
                sc.activation(osb[0:2, s], redLre[0:2, s], AF.Copy)
                sc.activation(osb[0:2, HALF + s.start:HALF + s.stop],
                              redLim[0:2, s], AF.Copy)
                yield

            def v_pass(i, s):
                def W(tag, dt=F32):
                    return wk.tile([128, CH], dt, tag=f"{tag}c{i}",
                                   name=f"t{tag}c{i}")

                pa_V = P("pa_V", "paV")
                mm(pa_V[:, s], lhs_VA, rhs_pa[:, s], start=True, stop=True)
                pab_V = P("pab_V", "pabV")
                mm(pab_V[:, s], lhs_VAB, rhs_pab[:, s], start=True, stop=True)
                yield
                e_a2v = W("v01")
                sc.activation(e_a2v[:], pa_V[:, s], AF.Exp)  # zs^4 e^{pa(z)}
                yield
                omz4v = W("v02")
                sc.activation(omz4v[:], zs4b[:, s], AF.Copy, bias=1.0,
                              scale=nu4V_c)
                yield
                Xv = W("v03")
                gp.tensor_tensor(Xv[:], omz4v[:], e_a2v[:], OP.mult)
                yield
                n2v = W("v04")
                v._custom_dve(_AFFSQ, out=n2v[:], in0=Xv[:], s0=1.0,
                              s1=EPS, imm2=EPS2)
                yield
                rn2v = W("v05")
                v.reciprocal_approx_fast(rn2v[:], n2v[:])
                yield
                t2 = W("v06")
                v._custom_dve(_MULAFF, out=t2[:], in0=rn2v[:], in1=c1b[:, s],
                              s0=u4V_c, s1=0.0)
                yield
                t2Xp = W("v07")
                v._custom_dve(_MULAFF, out=t2Xp[:], in0=t2[:], in1=Xv[:],
                              s0=1.0, s1=EPS)
                yield
                r2v2 = W("v08")
                v._custom_dve(_R2FULLP, out=r2v2[:], in0=t2Xp[:], in1=t2[:],
                              s0=1.0 + EPS, imm2=EPS2)
                yield
                lnr2v = W("v09")
                sc.activation(lnr2v[:], r2v2[:], AF.Ln)
                yield
                rv = W("v10", dt=BF16)
                sc.activation(rv[:], lnr2v[:], AF.Exp, scale=0.5)
                yield
                rlg2 = W("v11")
                v._custom_dve(_ABSD_ADD, out=rlg2[:], in0=t2Xp[:], in1=rv[:],
                              s0=1.0 + EPS)
                yield
                lnrlg2 = W("v12")
                sc.activation(lnrlg2[:], rlg2[:], AF.Ln)
                yield
                base2v = W("v13")
                v.tensor_tensor(base2v[:], pab_V[:, s], lnr2v[:], OP.subtract)
                yield
                lnglg2 = W("v14")
                gp.tensor_tensor(lnglg2[:], base2v[:], lnrlg2[:], OP.add)
                yield
                SSv = W("v15")
                sc.activation(SSv[:], lnglg2[:], AF.Exp, bias=nhln2[:, 0:1],
                              scale=0.5)
                yield
                rcpl2 = W("v16")
                v.reciprocal_approx_fast(rcpl2[:], rlg2[:])
                yield
                h1v = W("v17")
                v._custom_dve(_ONEP_MUL, out=h1v[:], in0=rcpl2[:], in1=t2[:],
                              s0=EPS)
                yield
                TTv = W("v18")
                gp.tensor_tensor(TTv[:], SSv[:], h1v[:], OP.mult)
                yield
                SSsv = W("v19")
                v._custom_dve(_SGN_LE, out=SSsv[:], in0=SSv[:], in1=t2Xp[:],
                              s0=1.0 + EPS)
                yield
                P2 = W("v20")
                v.tensor_tensor(P2[:], SSsv[:], TTv[:], OP.max)
                yield
                mnv = W("v21")
                v.tensor_tensor(mnv[:], SSsv[:], TTv[:], OP.min)
                yield
                M2 = W("v22")
                sc.activation(M2[:], mnv[:], AF.Abs)
                yield
                zd = W("v23")
                sc.activation(zd[:], zs2b[:, s], AF.Copy, bias=EPS,
                              scale=u2V_c)                    # z^2 + eps
                sc.activation(invz2[:, s], lnz2[:, s], AF.Exp, scale=-1.0)
                yield
                ndn = W("v24")
                v._custom_dve(_AFFSQ, out=ndn[:], in0=zd[:], s0=1.0,
                              s1=0.0, imm2=EPS2)
                yield
                rnd = W("v25")
                v.reciprocal_approx_fast(rnd[:], ndn[:])
                yield
                P2m = W("v26")
                v.tensor_tensor(P2m[:], P2[:], zsb[:, s], OP.subtract)
                yield
                A12 = W("v27")
                gp.tensor_tensor(A12[:], P2m[:], zd[:], OP.mult)
                yield
                igrev = W("v28", dt=BF16)
                v.tensor_tensor(igrev[:], A12[:], rnd[:], OP.mult)
                yield
                B1 = W("v29")
                gp.tensor_tensor(B1[:], M2[:], zd[:], OP.mult)
                yield
                B3 = W("v30")
                v.scalar_tensor_tensor(B3[:], P2m[:], EPS, B1[:], OP.mult,
                                       OP.add)
                yield
                igimv = W("v31", dt=BF16)
                gp.tensor_tensor(igimv[:], B3[:], rnd[:], OP.mult)
                yield
                redVre = P("redVre", "ps12")
                mm(redVre[0:2, s], diag2, invz2[:, s], start=True, stop=False)
                mm(redVre[0:2, s], wVre_c, igrev[:], start=False, stop=True)
                redVim = P("redVim", "zsb")
                mm(redVim[0:2, s], wVim_c, igimv[:], start=True, stop=True)
                yield
                sc.activation(osb[0:2, 2 * HALF + s.start:2 * HALF + s.stop],
                              redVre[0:2, s], AF.Copy)
                sc.activation(osb[0:2, 3 * HALF + s.start:3 * HALF + s.stop],
                              redVim[0:2, s], AF.Copy)
                yield

            g0 = [l_pass(0, CHUNKS[0]), v_pass(0, CHUNKS[0])]
            g1 = [l_pass(1, CHUNKS[1]), v_pass(1, CHUNKS[1])]
            for _ in range(4):
                g0 = [g for g in g0 if next(g, "end") != "end"]
            live = g0 + g1
            while live:
                live = [g for g in live if next(g, "end") != "end"]

            # ---------------- output ----------------
            nc.sync.dma_start(
                out=out_d[0:2, :].rearrange("a (b n) -> b a n", b=2),
                in_=osb[:, 0:2 * HALF].rearrange("b (a n) -> b a n", a=2))
            nc.sync.dma_start(
                out=out_d[2:4, :].rearrange("a (b n) -> b a n", b=2),
                in_=osb[:, 2 * HALF:].rearrange("b (a n) -> b a n", a=2))

    return nc


_NC_CACHE = {}


def _restrict_act_tables(nc):
    """Monkeypatch table-set selection to the one set that serves every
    activation this kernel uses (exp/ln/square/copy) so the steady state has
    zero ACT_TABLE_LOADs."""
    import types
    from concourse.hw_specs import get_activation_tables

    def _patched(self):
        tables = [(k, (v if k == "natural_log_exp_and_others" else set()))
                  for k, v in get_activation_tables(self.m.arch).items()]
        bacc._bass_rust.insert_act_table_loads(self, tables)

    nc.insert_act_table_loads = types.MethodType(_patched, nc)


def kernel(a, b, zs):
    a = np.asarray(a, dtype=np.float32)
    b = np.asarray(b, dtype=np.float32)
    zs = np.asarray(zs, dtype=np.float32)
    if "nc" not in _NC_CACHE:
        nc0 = build_nc()
        _restrict_act_tables(nc0)
        nc0.finalize()
        _NC_CACHE["nc"] = nc0
    nc = _NC_CACHE["nc"]
    in_maps = [
        {"a": a, "b": b, "zs": zs[i * BLOC: (i + 1) * BLOC].copy()}
        for i in range(NCORES)
    ]
    res = bass_utils.run_bass_kernel_spmd(nc, in_maps, core_ids=list(range(NCORES)))
    out = np.concatenate([res.results[i]["out"] for i in range(NCORES)], axis=1)
    return out.astype(np.float32)


if __name__ == "__main__":
    rng = np.random.default_rng(0)
    out = kernel(
        rng.standard_normal(5).astype(np.float32),
        rng.standard_normal(5).astype(np.float32),
        (0.02 + 0.975 * rng.random(8192)).astype(np.float32),
    )
    print(out.shape, out.dtype, out[:, :3])
